# revision 1
# baseline (speedup 1.0000x reference)
"""Bass/Tile kernel for nn_BinaryClassifierChain on 8 trn2 cores.

Math (per reference.py):
  wc   = softmax(word_class_features, axis=0)            # over batch dim
  base = concat([features, wc], -1)                      # [B, W, 1088]
  L    = base @ W[:, :1088].T + b                        # [B, W, 32]
  chain: p_i = sigmoid(L_i + sum_{j<i} Wbin[i, j] p_j)   # Wbin = W[:, 1088:]

Sharding: pure data-parallel over the words dim (1024 = 8 x 128).  The
softmax couples the batch dim, which stays intact per shard; words are
independent.

Per-core dataflow (v2):
  - features f32 --SWDGE cast-DMA--> X bf16 [128 w, 4 b, 1024 d] tiles
  - PE transpose [128,128] blocks -> psum bf16 [128, 512] (one k-chunk,
    4 batches) -> DVE/ACT copy to SBUF X^T tiles
  - PE matmul (W^T stationary, N=512 tokens) -> psum [32, 512] f32
  - ACT bias-add copy -> [32, 512] f32 SBUF; PE corner transpose
    4x[32,128] -> psum [128, 128] -> one ACT copy into L (bin-major)
  - wc: softmax on chip -> bf16 [b, w, c] to DRAM scratch -> one big
    xbar DMA transpose -> WCT [c, tok] -> last matmul k-chunk
  - chain: scalar_tensor_tensor MACs on DVE over [128, 64] contiguous
    slices (L/P bin-major [128, 32, 64]), sigmoids on ACT
  - pack P -> token-major PK on GpSimd, one store
"""

import sys

sys.path.insert(0, "/opt/trn_rl_repo")

import numpy as np
import orjson

import concourse.bass as bass
import concourse.mybir as mybir
import concourse.tile as tile
from concourse import masks
from concourse.bass_utils import run_bass_kernel_spmd

F32 = mybir.dt.float32
BF16 = mybir.dt.bfloat16
AF = mybir.ActivationFunctionType
ALU = mybir.AluOpType

B = 64          # batch
NWALL = 1024    # total words
NCORES = 8
NW = NWALL // NCORES  # 128 words per core
D = 1024        # embed dim
C = 64          # word classes
NB = 32         # bin features
DIN = D + C + NB  # 1120
GRP = 4         # batches per matmul group (4 * 128 words = 512 tokens)
NGRP = B // GRP

# how many of the 8 per-group evac copies go to DVE (rest go to ACT)
EVAC_DVE = 2


def _split_multiwait_json(raw: bytes) -> bytes:
    """walrus in this container only accepts 1 sync-wait per most
    instructions; Tile's final drain (and some others) carry several.
    Move extras onto preceding EventSemaphore carriers (2 waits each) on
    the same engine."""
    bir = orjson.loads(raw)
    for fn in bir["functions"]:
        for blk in fn["blocks"]:
            out = []
            for ins in blk["instructions"]:
                si = ins.get("sync_info")
                waits = (si or {}).get("on_wait") or []
                if len(waits) > 1:
                    extra = waits[:-1]
                    for k in range(0, len(extra), 2):
                        out.append(
                            {
                                "debug": ins.get("debug", 0),
                                "engine": ins["engine"],
                                "ins": [],
                                "outs": [],
                                "name": f"{ins['name']}_sw{k}",
                                "opcode": "EventSemaphore",
                                "sync_info": {
                                    "on_update": [],
                                    "on_wait": extra[k : k + 2],
                                },
                            }
                        )
                    si["on_wait"] = [waits[-1]]
                out.append(ins)
            blk["instructions"] = out
    return orjson.dumps(bir)


def build_program():
    nc = bass.Bass("TRN2", target_bir_lowering=False, debug=False)

    feat = nc.dram_tensor("feat", [B, NW, D], F32, kind="ExternalInput")
    wc = nc.dram_tensor("wc", [B, NW, C], F32, kind="ExternalInput")
    Wt = nc.dram_tensor("W", [NB, DIN], F32, kind="ExternalInput")
    bt = nc.dram_tensor("b", [NB], F32, kind="ExternalInput")
    out = nc.dram_tensor("out", [B, NW, NB], F32, kind="ExternalOutput")
    # DRAM scratch for the softmaxed wc in token-major layout, padded to
    # 128 classes so the big xbar transpose is legal.  The pad columns are
    # never written (garbage), but the transposed pad rows are never read.
    wcnd = nc.dram_tensor("wcnd", [B, NW, 128], BF16, kind="ExternalOutput")

    with tile.TileContext(nc) as tc:
        with (
            tc.tile_pool(name="const", bufs=1) as constp,
            tc.tile_pool(name="x2", bufs=3) as x2p,
            tc.tile_pool(name="xt", bufs=2) as xtp,
            tc.tile_pool(name="blt", bufs=2) as bltp,
            tc.tile_pool(name="lp", bufs=1) as lpp,
            tc.tile_pool(name="tp", bufs=3, space="PSUM") as tpp,
            tc.tile_pool(name="mmps", bufs=2, space="PSUM") as mmpsp,
            tc.tile_pool(name="petps", bufs=2, space="PSUM") as petpsp,
        ):
            # ---------------- prep ----------------
            ident = constp.tile([128, 128], BF16)
            masks.make_identity(nc, ident[:])
            identf = constp.tile([NB, NB], F32)
            masks.make_identity(nc, identf[:])

            b_sb = constp.tile([NB, 1], F32)
            nc.sync.dma_start(b_sb[:], bt.ap().unsqueeze(1))

            # W cast to bf16, padded to 1152 cols so 128-col xbar chunks cover it
            wbf = constp.tile([NB, 1152], BF16)
            nc.gpsimd.memset(wbf[:], 0.0)
            nc.gpsimd.dma_start(wbf[:, 0:DIN], Wt.ap())
            # transpose 9 chunks of 128 cols -> WT[128, 9, 32]
            wtr = constp.tile([128, 9, NB], BF16)
            for k in range(9):
                nc.sync.dma_start(
                    wtr[:, k, :], wbf[:, k * 128 : (k + 1) * 128], transpose=True
                )

            # replicate Wbin (f32) to all partitions via k=1 PE matmul
            # broadcast, through the corner-turn psum pool (8 x N=128)
            wbin1 = constp.tile([1, NB * NB], F32)
            nc.sync.dma_start(wbin1[:], Wt.ap()[:, D + C : DIN].unsqueeze(0))
            ones1 = constp.tile([1, 128], F32)
            nc.gpsimd.memset(ones1[:], 1.0)
            wrep = constp.tile([128, NB * NB], F32)
            for h in range(8):
                wps = petpsp.tile([128, 128], F32, tag="pet")
                nc.tensor.matmul(
                    wps[:], ones1[:], wbin1[:, h * 128 : (h + 1) * 128],
                    start=True, stop=True,
                )
                nc.vector.tensor_copy(wrep[:, h * 128 : (h + 1) * 128], wps[:])

            # WCT must outlive the softmax scratch scope
            wct = constp.tile([128, B * NW], BF16)

            # ---------------- softmax over batch ----------------
            with tc.tile_pool(name="soft", bufs=1) as softp:
                wcs = softp.tile([128, B, C], F32)
                nc.sync.dma_start(wcs[:], wc.ap().rearrange("b p c -> p b c"))
                ex = softp.tile([128, B, C], F32)
                nc.scalar.activation(ex[:], wcs[:], AF.Exp)
                acc = softp.tile([128, B // 2, C], F32)
                nc.vector.tensor_add(
                    acc[:], ex[:, 0 : B // 2, :], ex[:, B // 2 : B, :]
                )
                h = B // 4
                while h >= 1:
                    nc.vector.tensor_add(
                        acc[:, 0:h, :], acc[:, 0:h, :], acc[:, h : 2 * h, :]
                    )
                    h //= 2
                rec = softp.tile([128, C], F32)
                nc.vector.reciprocal(rec[:], acc[:, 0, :])
                wcn = softp.tile([128, B, C], BF16)
                nc.gpsimd.tensor_tensor(
                    wcn[:],
                    ex[:],
                    rec[:].unsqueeze(1).broadcast_to([128, B, C]),
                    op=ALU.mult,
                )
                # token-major store (real 64 classes only), then one big
                # DRAM->SBUF xbar transpose to [c, tok]
                nc.sync.dma_start(
                    wcnd.ap()[:, :, 0:C].rearrange("b p c -> p b c"), wcn[:]
                )
                nc.sync.dma_start(
                    wct[:],
                    wcnd.ap().rearrange("b p c -> (b p) c"),
                    transpose=True,
                )

            # ---------------- main matmul pipeline ----------------
            # L, P in token-major (AoS) layout [128, B batches, NB bins]
            L = lpp.tile([128, B, NB], F32)
            P = lpp.tile([128, B, NB], BF16)
            tmp = lpp.tile([128, B, NB], BF16)
            corr = lpp.tile([128, B], F32)

            for g in range(NGRP):
                b0 = g * GRP
                x2 = x2p.tile([128, GRP, D], BF16, tag="x2")
                nc.gpsimd.dma_start(
                    x2[:], feat.ap()[b0 : b0 + GRP, :, :].rearrange("b p d -> p b d")
                )
                xts = xtp.tile([128, 8, GRP * 128], BF16, tag="xt")
                for kh in range(4):
                    pt = tpp.tile([128, 2, GRP * 128], BF16, tag="xtps")
                    for kk in range(2):
                        k = kh * 2 + kk
                        for bi in range(GRP):
                            nc.tensor.transpose(
                                pt[:, kk, bi * 128 : (bi + 1) * 128],
                                x2[:, bi, k * 128 : (k + 1) * 128],
                                ident[:],
                            )
                    if (g * 4 + kh) % 4 == 0:
                        nc.vector.tensor_copy(xts[:, kh * 2 : kh * 2 + 2, :], pt[:])
                    else:
                        nc.scalar.copy(xts[:, kh * 2 : kh * 2 + 2, :], pt[:])
                ps = mmpsp.tile([NB, 512], F32, tag="mm")
                for k in range(8):
                    nc.tensor.matmul(
                        ps[:], wtr[:, k, :], xts[:, k, :],
                        start=(k == 0), stop=False,
                    )
                nc.tensor.matmul(
                    ps[:],
                    wtr[0:C, 8, :],
                    wct[0:C, b0 * 128 : (b0 + GRP) * 128],
                    start=False, stop=True,
                )
                blt = bltp.tile([NB, 512], F32, tag="blt")
                nc.scalar.activation(
                    blt[:], ps[:], AF.Identity, bias=b_sb[:, 0:1], scale=1.0
                )
                # corner turn: 4 x [32,128] -> one [128, 4*32] psum, one copy
                ptc = petpsp.tile([128, 128], F32, tag="pet")
                for q in range(GRP):
                    nc.tensor.transpose(
                        ptc[:, q * NB : (q + 1) * NB],
                        blt[:, q * 128 : (q + 1) * 128],
                        identf[:],
                    )
                nc.scalar.copy(L[:, b0 : b0 + GRP, :], ptc[:])

            # ---------------- sigmoid chain (2 token-halves for overlap) ----
            wrepb = constp.tile([128, NB * NB], BF16)
            nc.vector.tensor_copy(wrepb[:], wrep[:])
            BH = B // 2
            for i in range(NB):
                for h in range(2):
                    bs = slice(h * BH, (h + 1) * BH)
                    if i > 0:
                        wrow = wrepb[:, i * NB : i * NB + i]
                        nc.vector.tensor_mul(
                            tmp[:, bs, 0:i],
                            P[:, bs, 0:i],
                            wrow.unsqueeze(1).broadcast_to([128, BH, i]),
                        )
                        nc.vector.reduce_sum(
                            corr[:, bs], tmp[:, bs, 0:i], axis=mybir.AxisListType.X
                        )
                        nc.vector.scalar_tensor_tensor(
                            L[:, bs, i], corr[:, bs], 1.0, L[:, bs, i],
                            op0=ALU.mult, op1=ALU.add,
                        )
                    nc.scalar.activation(P[:, bs, i], L[:, bs, i], AF.Sigmoid)

            # store with bf16 -> f32 cast on the SWDGE path
            nc.gpsimd.dma_start(out.ap().rearrange("b p i -> p b i"), P[:])

    orig = nc.to_json_bytes
    nc.to_json_bytes = lambda: _split_multiwait_json(orig())
    return nc


_PROG = None


def _get_prog():
    global _PROG
    if _PROG is None:
        _PROG = build_program()
    return _PROG


def kernel(features, word_class_features, W, b, trace=False, tmpdir=None):
    features = np.ascontiguousarray(features, dtype=np.float32)
    word_class_features = np.ascontiguousarray(word_class_features, dtype=np.float32)
    W = np.ascontiguousarray(W, dtype=np.float32)
    b = np.ascontiguousarray(b, dtype=np.float32)

    nc = _get_prog()
    in_maps = []
    for c in range(NCORES):
        sl = slice(c * NW, (c + 1) * NW)
        in_maps.append(
            {
                "feat": np.ascontiguousarray(features[:, sl, :]),
                "wc": np.ascontiguousarray(word_class_features[:, sl, :]),
                "W": W,
                "b": b,
            }
        )
    res = run_bass_kernel_spmd(
        nc, in_maps, core_ids=list(range(NCORES)), trace=trace, tmpdir=tmpdir
    )
    outp = np.concatenate([res.results[c]["out"] for c in range(NCORES)], axis=1)
    kernel._last_result = res
    return outp



# revision 2
# speedup vs baseline: 1.1337x; 1.1337x over previous
"""Bass/Tile kernel for nn_BinaryClassifierChain on 8 trn2 cores (v3).

Math (per reference.py):
  wc   = softmax(word_class_features, axis=0)            # over batch dim
  base = concat([features, wc], -1)                      # [B, W, 1088]
  L    = base @ W[:, :1088].T + b                        # [B, W, 32]
  chain: p_i = sigmoid(L_i + sum_{j<i} Wbin[i, j] p_j)   # Wbin = W[:, 1088:]

Sharding: pure data-parallel over the words dim (1024 = 8 x 128); the
batch-softmax stays intact per shard.

v3 changes vs v2 baseline (267us):
  - W^T / bias / chain-weight prep moved to numpy host code (tiny tensors).
  - wc softmax transposed on-chip via PE (4 small transposes per group)
    feeding the matmul as a 9th k-chunk -- kills the DRAM round trip and
    the 18.7us xbar DMA transpose of v2.
  - Chain rewrite: logits and probs share one token-major tile Z; the
    host-built vrows matrix carries Wbin rows with a unit diagonal so a
    single mul+reduce per bin covers both the correction and the +L term
    (no separate add).  Chunk 0 (batches 0-31) is emitted interleaved
    into the group loop so its DVE work hides under the matmul phase;
    chunk 1 runs as two interleaved 16-batch half-chains to hide the
    serial per-bin latency in the tail.
  - Output DRAM tensor is bf16 stored on the idle sync HWDGE queue per
    chunk (host casts back to f32); v2 spent ~20us on a serial SWDGE
    cast store at the end.
  - 2-bank PSUM transpose tiles -> one [128,2048] evacuation per 4
    k-chunks (halves the ACT/DVE per-instruction overhead of the X^T
    evacuation).
  - Deeper feature-DMA prefetch (bufs=5).
"""

import sys

sys.path.insert(0, "/opt/trn_rl_repo")

import numpy as np
import orjson
import ml_dtypes

import concourse.bass as bass
import concourse.mybir as mybir
import concourse.tile as tile
from concourse import masks
from concourse.bass_utils import run_bass_kernel_spmd

F32 = mybir.dt.float32
BF16 = mybir.dt.bfloat16
AF = mybir.ActivationFunctionType
ALU = mybir.AluOpType
AX = mybir.AxisListType

B = 64          # batch
NWALL = 1024    # total words
NCORES = 8
NW = NWALL // NCORES  # 128 words per core
D = 1024        # embed dim
C = 64          # word classes
NB = 32         # bin features
DIN = D + C + NB  # 1120
GRP = 4         # batches per matmul group (4 * 128 words = 512 tokens)
NGRP = B // GRP

# chain chunking: chunk0 = batches [0, CH0); rest is the tail pair
CH0 = 32
# which of the 2 big X^T evacuations per group go to DVE instead of ACT
# (list of (group, half) pairs is overkill; use: DVE gets half 1 for
# groups < N_DVE_EVAC)
N_DVE_EVAC = 10


def _split_multiwait_json(raw: bytes) -> bytes:
    """walrus in this container only accepts 1 sync-wait per most
    instructions; Tile's final drain (and some others) carry several.
    Move extras onto preceding EventSemaphore carriers (2 waits each) on
    the same engine."""
    bir = orjson.loads(raw)
    for fn in bir["functions"]:
        for blk in fn["blocks"]:
            out = []
            for ins in blk["instructions"]:
                si = ins.get("sync_info")
                waits = (si or {}).get("on_wait") or []
                if len(waits) > 1:
                    extra = waits[:-1]
                    for k in range(0, len(extra), 2):
                        out.append(
                            {
                                "debug": ins.get("debug", 0),
                                "engine": ins["engine"],
                                "ins": [],
                                "outs": [],
                                "name": f"{ins['name']}_sw{k}",
                                "opcode": "EventSemaphore",
                                "sync_info": {
                                    "on_update": [],
                                    "on_wait": extra[k : k + 2],
                                },
                            }
                        )
                    si["on_wait"] = [waits[-1]]
                out.append(ins)
            blk["instructions"] = out
    return orjson.dumps(bir)


def build_program():
    nc = bass.Bass("TRN2", target_bir_lowering=False, debug=False)

    feat = nc.dram_tensor("feat", [B, NW, D], F32, kind="ExternalInput")
    wc = nc.dram_tensor("wc", [B, NW, C], F32, kind="ExternalInput")
    wtrd = nc.dram_tensor("wtr", [128, 9, NB], BF16, kind="ExternalInput")
    vrd = nc.dram_tensor("vrows", [128, NB, NB], BF16, kind="ExternalInput")
    bt = nc.dram_tensor("b", [NB], F32, kind="ExternalInput")
    out = nc.dram_tensor("out", [B, NW, NB], BF16, kind="ExternalOutput")

    with tile.TileContext(nc) as tc:
        with (
            tc.tile_pool(name="const", bufs=1) as constp,
            tc.tile_pool(name="x2", bufs=5) as x2p,
            tc.tile_pool(name="xt", bufs=2) as xtp,
            tc.tile_pool(name="blt", bufs=2) as bltp,
            tc.tile_pool(name="tp", bufs=2, space="PSUM") as tpp,
            tc.tile_pool(name="wcps", bufs=1, space="PSUM") as wcpsp,
            tc.tile_pool(name="mmps", bufs=2, space="PSUM") as mmpsp,
            tc.tile_pool(name="petps", bufs=1, space="PSUM") as petpsp,
        ):
            # ---------------- prep (host-precomputed weights) ----------
            ident = constp.tile([128, 128], BF16)
            masks.make_identity(nc, ident[:])
            identf = constp.tile([NB, NB], F32)
            masks.make_identity(nc, identf[:])

            b_sb = constp.tile([NB, 1], F32)
            nc.sync.dma_start(b_sb[:], bt.ap().unsqueeze(1))
            wtr = constp.tile([128, 9, NB], BF16)
            nc.sync.dma_start(wtr[:], wtrd.ap())
            vr = constp.tile([128, NB, NB], BF16)
            nc.sync.dma_start(vr[:], vrd.ap())

            wcn = constp.tile([128, B, C], BF16)
            # token-major chain state: [words, batch, bins]; slot i holds
            # L_i until bin i's sigmoid overwrites it with p_i
            Z = constp.tile([128, B, NB], BF16)
            tmp0 = constp.tile([128, CH0, NB + 1], BF16)
            zc0 = constp.tile([128, CH0], F32)
            BH = (B - CH0) // 2
            tmp1 = constp.tile([128, BH, NB + 1], BF16)
            zc1 = constp.tile([128, BH], F32)
            tmp2 = constp.tile([128, BH, NB + 1], BF16)
            zc2 = constp.tile([128, BH], F32)

            # ---------------- softmax over batch ----------------
            with tc.tile_pool(name="soft", bufs=1) as softp:
                wcs = softp.tile([128, B, C], F32)
                nc.sync.dma_start(wcs[:], wc.ap().rearrange("b p c -> p b c"))
                ex = softp.tile([128, B, C], F32)
                nc.scalar.activation(ex[:], wcs[:], AF.Exp)
                acc = softp.tile([128, B // 2, C], F32)
                nc.vector.tensor_add(
                    acc[:], ex[:, 0 : B // 2, :], ex[:, B // 2 : B, :]
                )
                h = B // 4
                while h >= 1:
                    nc.vector.tensor_add(
                        acc[:, 0:h, :], acc[:, 0:h, :], acc[:, h : 2 * h, :]
                    )
                    h //= 2
                rec = softp.tile([128, C], F32)
                nc.vector.reciprocal(rec[:], acc[:, 0, :])
                nc.vector.tensor_mul(
                    wcn[:],
                    ex[:],
                    rec[:].unsqueeze(1).broadcast_to([128, B, C]),
                )

            # ---------------- chain helper ----------------
            def chain_bin(i, bs, tmp, zc):
                """One chain step for bins over batch slice bs."""
                nbt = bs.stop - bs.start
                if i == 0:
                    # Z slot 0 is L_0; vrow would be [1.0] -- sigmoid direct
                    nc.scalar.activation(Z[:, bs, 0], Z[:, bs, 0], AF.Sigmoid)
                    return
                nc.vector.tensor_mul(
                    tmp[:, :, 0 : i + 1],
                    Z[:, bs, 0 : i + 1],
                    vr[:, i, 0 : i + 1]
                    .unsqueeze(1)
                    .broadcast_to([128, nbt, i + 1]),
                )
                nc.vector.reduce_sum(zc[:, :], tmp[:, :, 0 : i + 1], axis=AX.X)
                nc.scalar.activation(Z[:, bs, i], zc[:, :], AF.Sigmoid)

            bs0 = slice(0, CH0)
            # chunk-0 bins are spread over groups 8..15, CH_PER_SLOT each,
            # at 4 interleave points inside the group body
            CH_SLOT_G0 = 8
            CH_PER_SLOT = 4

            def c0_bins_for(g, pos):
                """chain-chunk-0 bin indices to emit at position pos (0-3)
                inside group g's body."""
                if g < CH_SLOT_G0:
                    return []
                base = (g - CH_SLOT_G0) * CH_PER_SLOT
                per_pos = [1, 1, 1, 1]
                start = base + sum(per_pos[:pos])
                return list(range(start, start + per_pos[pos]))

            # ---------------- main matmul pipeline ----------------
            for g in range(NGRP):
                b0 = g * GRP
                x2 = x2p.tile([128, GRP, D], BF16, tag="x2")
                nc.gpsimd.dma_start(
                    x2[:], feat.ap()[b0 : b0 + GRP, :, :].rearrange("b p d -> p b d")
                )
                xts = xtp.tile([128, 9, 512], BF16, tag="xt")
                for kh in range(2):
                    pt = tpp.tile([128, 4, 512], BF16, tag="xtps")
                    for kk in range(4):
                        k = kh * 4 + kk
                        for bi in range(GRP):
                            nc.tensor.transpose(
                                pt[:, kk, bi * 128 : (bi + 1) * 128],
                                x2[:, bi, k * 128 : (k + 1) * 128],
                                ident[:],
                            )
                    if kh == 1 and g < N_DVE_EVAC:
                        nc.vector.tensor_copy(xts[:, kh * 4 : kh * 4 + 4, :], pt[:])
                    else:
                        nc.scalar.copy(xts[:, kh * 4 : kh * 4 + 4, :], pt[:])
                    for i in c0_bins_for(g, kh):
                        chain_bin(i, bs0, tmp0, zc0)

                # softmaxed wc as 9th k-chunk: transpose on chip
                wps = wcpsp.tile([64, 512], BF16, tag="wct")
                for bi in range(GRP):
                    nc.tensor.transpose(
                        wps[:, bi * 128 : (bi + 1) * 128],
                        wcn[:, b0 + bi, :],
                        ident[:],
                    )
                nc.vector.tensor_copy(xts[0:64, 8, :], wps[:])
                for i in c0_bins_for(g, 2):
                    chain_bin(i, bs0, tmp0, zc0)

                ps = mmpsp.tile([NB, 512], F32, tag="mm")
                for k in range(8):
                    nc.tensor.matmul(
                        ps[:], wtr[:, k, :], xts[:, k, :],
                        start=(k == 0), stop=False,
                    )
                nc.tensor.matmul(
                    ps[:], wtr[0:64, 8, :], xts[0:64, 8, :],
                    start=False, stop=True,
                )
                blt = bltp.tile([NB, 512], F32, tag="blt")
                nc.scalar.activation(
                    blt[:], ps[:], AF.Identity, bias=b_sb[:, 0:1], scale=1.0
                )
                # corner turn: 4 x [32,128] -> one [128, 4*32] psum, one copy
                ptc = petpsp.tile([128, 128], F32, tag="pet")
                for q in range(GRP):
                    nc.tensor.transpose(
                        ptc[:, q * NB : (q + 1) * NB],
                        blt[:, q * 128 : (q + 1) * 128],
                        identf[:],
                    )
                nc.scalar.copy(Z[:, b0 : b0 + GRP, :], ptc[:])
                for i in c0_bins_for(g, 3):
                    chain_bin(i, bs0, tmp0, zc0)

            # chunk-0 store (batches 0..CH0)
            nc.sync.dma_start(
                out.ap()[0:CH0, :, :].rearrange("b p i -> p b i"), Z[:, bs0, :]
            )

            # ---------------- tail: two interleaved half-chains ----------
            bsA = slice(CH0, CH0 + BH)
            bsB = slice(CH0 + BH, B)
            for i in range(NB):
                chain_bin(i, bsA, tmp1, zc1)
                chain_bin(i, bsB, tmp2, zc2)
            nc.sync.dma_start(
                out.ap()[CH0:B, :, :].rearrange("b p i -> p b i"),
                Z[:, CH0:B, :],
            )

    orig = nc.to_json_bytes
    nc.to_json_bytes = lambda: _split_multiwait_json(orig())
    return nc


_PROG = None


def _get_prog():
    global _PROG
    if _PROG is None:
        _PROG = build_program()
    return _PROG


def _host_weights(W, b):
    """Host-side prep of the tiny weight tensors."""
    W = np.asarray(W, dtype=np.float32)
    wtr = np.zeros((128, 9, NB), dtype=ml_dtypes.bfloat16)
    for k in range(8):
        wtr[:, k, :] = W[:, k * 128 : (k + 1) * 128].T.astype(ml_dtypes.bfloat16)
    wtr[0:64, 8, :] = W[:, D : D + C].T.astype(ml_dtypes.bfloat16)
    wbin = W[:, D + C : DIN]  # [32, 32]
    vr = np.zeros((NB, NB), dtype=np.float32)
    for i in range(NB):
        vr[i, :i] = wbin[i, :i]
        vr[i, i] = 1.0
    vrows = np.broadcast_to(
        vr.astype(ml_dtypes.bfloat16)[None], (128, NB, NB)
    ).copy()
    return wtr, vrows, np.ascontiguousarray(b, dtype=np.float32)


def kernel(features, word_class_features, W, b, trace=False, tmpdir=None):
    features = np.ascontiguousarray(features, dtype=np.float32)
    word_class_features = np.ascontiguousarray(word_class_features, dtype=np.float32)
    wtr, vrows, bf = _host_weights(W, b)

    nc = _get_prog()
    in_maps = []
    for c in range(NCORES):
        sl = slice(c * NW, (c + 1) * NW)
        in_maps.append(
            {
                "feat": np.ascontiguousarray(features[:, sl, :]),
                "wc": np.ascontiguousarray(word_class_features[:, sl, :]),
                "wtr": wtr,
                "vrows": vrows,
                "b": bf,
            }
        )
    res = run_bass_kernel_spmd(
        nc, in_maps, core_ids=list(range(NCORES)), trace=trace, tmpdir=tmpdir
    )
    outp = np.concatenate(
        [res.results[c]["out"].astype(np.float32) for c in range(NCORES)], axis=1
    )
    kernel._last_result = res
    return outp


# revision 3
# speedup vs baseline: 1.1687x; 1.0309x over previous
"""Bass/Tile kernel for nn_BinaryClassifierChain on 8 trn2 cores (v4).

Math (per reference.py):
  wc   = softmax(word_class_features, axis=0)            # over batch dim
  base = concat([features, wc], -1)                      # [B, W, 1088]
  L    = base @ W[:, :1088].T + b                        # [B, W, 32]
  chain: p_i = sigmoid(L_i + sum_{j<i} Wbin[i, j] p_j)   # Wbin = W[:, 1088:]

Sharding: pure data-parallel over the words dim (1024 = 8 x 128); the
batch-softmax stays intact per shard.

v4 vs v3: the v3 trace showed (a) the wc load starving ~60us behind the
feature stream on the shared SDMA engines, and (b) the gpsimd SWDGE
cast-load path capping at ~145 GB/s.  So:
  - features load as plain f32 on the two HWDGE rings (sync + scalar,
    alternating groups); PE transposes run in f32 (transpose_mode) and
    the psum->SBUF evacuation casts to bf16 for the matmul.
  - wc loads FIRST on the sync ring, before any feature traffic.
  - output stores moved to the now-idle gpsimd SWDGE queue.
  - chain: chunk0 = batches 0-31 interleaved into groups 8-15; tail =
    two interleaved 16-batch half-chains.
"""

import sys

sys.path.insert(0, "/opt/trn_rl_repo")

import numpy as np
import orjson
import ml_dtypes

import concourse.bass as bass
import concourse.mybir as mybir
import concourse.tile as tile
from concourse import masks
from concourse.bass_utils import run_bass_kernel_spmd

F32 = mybir.dt.float32
BF16 = mybir.dt.bfloat16
AF = mybir.ActivationFunctionType
ALU = mybir.AluOpType
AX = mybir.AxisListType

B = 64          # batch
NWALL = 1024    # total words
NCORES = 8
NW = NWALL // NCORES  # 128 words per core
D = 1024        # embed dim
C = 64          # word classes
NB = 32         # bin features
DIN = D + C + NB  # 1120
GRP = 4         # batches per matmul group (4 * 128 words = 512 tokens)
NGRP = B // GRP

CH0 = 32        # chain chunk 0 = batches [0, CH0)


def _split_multiwait_json(raw: bytes) -> bytes:
    """walrus in this container only accepts 1 sync-wait per most
    instructions; Tile's final drain (and some others) carry several.
    Move extras onto preceding EventSemaphore carriers (2 waits each) on
    the same engine."""
    bir = orjson.loads(raw)
    for fn in bir["functions"]:
        for blk in fn["blocks"]:
            out = []
            for ins in blk["instructions"]:
                si = ins.get("sync_info")
                waits = (si or {}).get("on_wait") or []
                if len(waits) > 1:
                    extra = waits[:-1]
                    for k in range(0, len(extra), 2):
                        out.append(
                            {
                                "debug": ins.get("debug", 0),
                                "engine": ins["engine"],
                                "ins": [],
                                "outs": [],
                                "name": f"{ins['name']}_sw{k}",
                                "opcode": "EventSemaphore",
                                "sync_info": {
                                    "on_update": [],
                                    "on_wait": extra[k : k + 2],
                                },
                            }
                        )
                    si["on_wait"] = [waits[-1]]
                out.append(ins)
            blk["instructions"] = out
    return orjson.dumps(bir)


def build_program():
    nc = bass.Bass("TRN2", target_bir_lowering=False, debug=False)

    feat = nc.dram_tensor("feat", [B, NW, D], F32, kind="ExternalInput")
    wc = nc.dram_tensor("wc", [B, NW, C], F32, kind="ExternalInput")
    wtrd = nc.dram_tensor("wtr", [128, 9, NB], BF16, kind="ExternalInput")
    vrd = nc.dram_tensor("vrows", [128, NB, NB], BF16, kind="ExternalInput")
    bt = nc.dram_tensor("b", [NB], F32, kind="ExternalInput")
    out = nc.dram_tensor("out", [B, NW, NB], BF16, kind="ExternalOutput")

    with tile.TileContext(nc) as tc:
        with (
            tc.tile_pool(name="const", bufs=1) as constp,
            tc.tile_pool(name="x2", bufs=4) as x2p,
            tc.tile_pool(name="xt", bufs=2) as xtp,
            tc.tile_pool(name="blt", bufs=2) as bltp,
            tc.tile_pool(name="tp", bufs=2, space="PSUM") as tpp,
            tc.tile_pool(name="wcps", bufs=1, space="PSUM") as wcpsp,
            tc.tile_pool(name="mmps", bufs=2, space="PSUM") as mmpsp,
            tc.tile_pool(name="petps", bufs=1, space="PSUM") as petpsp,
        ):
            # wc softmax input load comes FIRST on the sync ring so it is
            # not starved by the feature stream.
            wcs = constp.tile([128, B, C], F32)
            nc.sync.dma_start(wcs[:], wc.ap().rearrange("b p c -> p b c"))

            # ---------------- prep (host-precomputed weights) ----------
            ident = constp.tile([128, 128], BF16)
            masks.make_identity(nc, ident[:])
            identf = constp.tile([128, 128], F32)
            masks.make_identity(nc, identf[:])

            b_sb = constp.tile([NB, 1], F32)
            nc.scalar.dma_start(b_sb[:], bt.ap().unsqueeze(1))
            wtr = constp.tile([128, 9, NB], BF16)
            nc.scalar.dma_start(wtr[:], wtrd.ap())
            vr = constp.tile([128, NB, NB], BF16)
            nc.scalar.dma_start(vr[:], vrd.ap())

            wcn = constp.tile([128, B, C], BF16)
            # token-major chain state: [words, batch, bins]; slot i holds
            # L_i until bin i's sigmoid overwrites it with p_i
            Z = constp.tile([128, B, NB], BF16)
            tmp0 = constp.tile([128, CH0, NB + 1], BF16)
            zc0 = constp.tile([128, CH0], F32)
            BH = (B - CH0) // 2
            tmp1 = constp.tile([128, BH, NB + 1], BF16)
            zc1 = constp.tile([128, BH], F32)
            tmp2 = constp.tile([128, BH, NB + 1], BF16)
            zc2 = constp.tile([128, BH], F32)

            # ---------------- softmax over batch ----------------
            with tc.tile_pool(name="soft", bufs=1) as softp:
                ex = softp.tile([128, B, C], F32)
                nc.scalar.activation(ex[:], wcs[:], AF.Exp)
                acc = softp.tile([128, B // 2, C], F32)
                nc.vector.tensor_add(
                    acc[:], ex[:, 0 : B // 2, :], ex[:, B // 2 : B, :]
                )
                h = B // 4
                while h >= 1:
                    nc.vector.tensor_add(
                        acc[:, 0:h, :], acc[:, 0:h, :], acc[:, h : 2 * h, :]
                    )
                    h //= 2
                rec = softp.tile([128, C], F32)
                nc.vector.reciprocal(rec[:], acc[:, 0, :])
                nc.vector.tensor_mul(
                    wcn[:],
                    ex[:],
                    rec[:].unsqueeze(1).broadcast_to([128, B, C]),
                )

            # ---------------- chain helper ----------------
            def chain_bin(i, bs, tmp, zc):
                nbt = bs.stop - bs.start
                if i == 0:
                    nc.scalar.activation(Z[:, bs, 0], Z[:, bs, 0], AF.Sigmoid)
                    return
                nc.vector.tensor_mul(
                    tmp[:, :, 0 : i + 1],
                    Z[:, bs, 0 : i + 1],
                    vr[:, i, 0 : i + 1]
                    .unsqueeze(1)
                    .broadcast_to([128, nbt, i + 1]),
                )
                nc.vector.reduce_sum(zc[:, :], tmp[:, :, 0 : i + 1], axis=AX.X)
                nc.scalar.activation(Z[:, bs, i], zc[:, :], AF.Sigmoid)

            bs0 = slice(0, CH0)
            CH_SLOT_G0 = 8   # chunk-0 bins spread over groups 8..15

            def c0_bins_for(g, pos):
                if g < CH_SLOT_G0:
                    return []
                base = (g - CH_SLOT_G0) * 4
                return [base + pos] if pos < 4 else []

            # ---------------- main matmul pipeline ----------------
            for g in range(NGRP):
                b0 = g * GRP
                x2 = x2p.tile([128, GRP, D], F32, tag="x2")
                ldq = nc.sync if g % 2 == 0 else nc.scalar
                ldq.dma_start(
                    x2[:], feat.ap()[b0 : b0 + GRP, :, :].rearrange("b p d -> p b d")
                )
                xts = xtp.tile([128, 9, 512], BF16, tag="xt")
                for kh in range(4):
                    pt = tpp.tile([128, 2, 512], F32, tag="xtps")
                    for kk in range(2):
                        k = kh * 2 + kk
                        for bi in range(GRP):
                            nc.tensor.transpose(
                                pt[:, kk, bi * 128 : (bi + 1) * 128],
                                x2[:, bi, k * 128 : (k + 1) * 128],
                                identf[:],
                            )
                    # cast-evacuation f32 psum -> bf16 SBUF, split ACT/DVE
                    if kh % 2 == 0:
                        nc.scalar.copy(xts[:, kh * 2 : kh * 2 + 2, :], pt[:])
                    else:
                        nc.vector.tensor_copy(xts[:, kh * 2 : kh * 2 + 2, :], pt[:])
                    if kh < 2:
                        for i in c0_bins_for(g, kh):
                            chain_bin(i, bs0, tmp0, zc0)

                # softmaxed wc as 9th k-chunk: transpose on chip
                wps = wcpsp.tile([64, 512], BF16, tag="wct")
                for bi in range(GRP):
                    nc.tensor.transpose(
                        wps[:, bi * 128 : (bi + 1) * 128],
                        wcn[:, b0 + bi, :],
                        ident[:],
                    )
                nc.scalar.copy(xts[0:64, 8, :], wps[:])
                for i in c0_bins_for(g, 2):
                    chain_bin(i, bs0, tmp0, zc0)

                ps = mmpsp.tile([NB, 512], F32, tag="mm")
                for k in range(8):
                    nc.tensor.matmul(
                        ps[:], wtr[:, k, :], xts[:, k, :],
                        start=(k == 0), stop=False,
                    )
                nc.tensor.matmul(
                    ps[:], wtr[0:64, 8, :], xts[0:64, 8, :],
                    start=False, stop=True,
                )
                blt = bltp.tile([NB, 512], F32, tag="blt")
                nc.scalar.activation(
                    blt[:], ps[:], AF.Identity, bias=b_sb[:, 0:1], scale=1.0
                )
                # corner turn: 4 x [32,128] -> one [128, 4*32] psum, one copy
                ptc = petpsp.tile([128, 128], F32, tag="pet")
                for q in range(GRP):
                    nc.tensor.transpose(
                        ptc[:, q * NB : (q + 1) * NB],
                        blt[:, q * 128 : (q + 1) * 128],
                        identf[0:NB, 0:NB],
                    )
                nc.vector.tensor_copy(Z[:, b0 : b0 + GRP, :], ptc[:])
                for i in c0_bins_for(g, 3):
                    chain_bin(i, bs0, tmp0, zc0)

            # chunk-0 store (batches 0..CH0) on the idle SWDGE queue
            nc.gpsimd.dma_start(
                out.ap()[0:CH0, :, :].rearrange("b p i -> p b i"), Z[:, bs0, :]
            )

            # ---------------- tail: two interleaved half-chains ----------
            bsA = slice(CH0, CH0 + BH)
            bsB = slice(CH0 + BH, B)
            for i in range(NB):
                chain_bin(i, bsA, tmp1, zc1)
                chain_bin(i, bsB, tmp2, zc2)
            nc.gpsimd.dma_start(
                out.ap()[CH0:B, :, :].rearrange("b p i -> p b i"),
                Z[:, CH0:B, :],
            )

    orig = nc.to_json_bytes
    nc.to_json_bytes = lambda: _split_multiwait_json(orig())
    return nc


_PROG = None


def _get_prog():
    global _PROG
    if _PROG is None:
        _PROG = build_program()
    return _PROG


def _host_weights(W, b):
    """Host-side prep of the tiny weight tensors."""
    W = np.asarray(W, dtype=np.float32)
    wtr = np.zeros((128, 9, NB), dtype=ml_dtypes.bfloat16)
    for k in range(8):
        wtr[:, k, :] = W[:, k * 128 : (k + 1) * 128].T.astype(ml_dtypes.bfloat16)
    wtr[0:64, 8, :] = W[:, D : D + C].T.astype(ml_dtypes.bfloat16)
    wbin = W[:, D + C : DIN]  # [32, 32]
    vr = np.zeros((NB, NB), dtype=np.float32)
    for i in range(NB):
        vr[i, :i] = wbin[i, :i]
        vr[i, i] = 1.0
    vrows = np.broadcast_to(
        vr.astype(ml_dtypes.bfloat16)[None], (128, NB, NB)
    ).copy()
    return wtr, vrows, np.ascontiguousarray(b, dtype=np.float32)


def kernel(features, word_class_features, W, b, trace=False, tmpdir=None):
    features = np.ascontiguousarray(features, dtype=np.float32)
    word_class_features = np.ascontiguousarray(word_class_features, dtype=np.float32)
    wtr, vrows, bf = _host_weights(W, b)

    nc = _get_prog()
    in_maps = []
    for c in range(NCORES):
        sl = slice(c * NW, (c + 1) * NW)
        in_maps.append(
            {
                "feat": np.ascontiguousarray(features[:, sl, :]),
                "wc": np.ascontiguousarray(word_class_features[:, sl, :]),
                "wtr": wtr,
                "vrows": vrows,
                "b": bf,
            }
        )
    res = run_bass_kernel_spmd(
        nc, in_maps, core_ids=list(range(NCORES)), trace=trace, tmpdir=tmpdir
    )
    outp = np.concatenate(
        [res.results[c]["out"].astype(np.float32) for c in range(NCORES)], axis=1
    )
    kernel._last_result = res
    return outp
